# revision 11
# baseline (speedup 1.0000x reference)
"""Causal self-attention Trainium2 kernel (B=2, T=2048, C=1024, H=16).

Returns (y, att_scores) like the reference:
    qh/kh/vh = split_heads(x @ W.T + b)
    att      = (qh @ kh^T) / sqrt(HD)
    scores   = where(tril, att, -inf) + attn_bias          # output 2
    y        = softmax(scores) @ vh -> merge -> @ Wp.T + bp  # output 1

Sharding (8 cores): core c owns batch b = c//4 and heads 4*(c%4)..+4
(data parallel on B x tensor parallel on H).  Each core runs its 4 heads'
projections (host-sliced weight rows), full causal attention, and a
partial output projection; the host sums the four per-batch partials
(the output projection contracts over head channels) and adds bp.

Device-side notes:
  - q/k/v ship pre-transposed [C, T] so the C contraction lies on SBUF
    partitions; weights ship pre-sliced/pre-transposed.
  - scores are written packed (lower-triangle 128-row strips only); the
    constant -inf upper triangle is filled host-side.  attn_bias is added
    during its own DMA (SWDGE accum_op=add), costing no engine time.
  - P = exp(scores) rides exp(-inf)=0 for the causal mask; P tiles are
    transposed 128x128 on the PE to feed P^T into the AV matmul.  V is
    augmented with a ones column so the AV matmul also emits the softmax
    row sums (unnormalized-softmax trick); 1/l is broadcast and applied
    when copying y^T out of PSUM, right before the output projection.
  - matmuls run in float32r (PE truncates fp32 operands to ~FP22; same
    read bytes, 4x the throughput of true fp32).  Set USE_FP32R=False to
    fall back to exact fp32 matmuls.
"""

import os
import sys
from contextlib import ExitStack

import numpy as np

_TRN_REPO = "/opt/trn_rl_repo"
if os.path.isdir(_TRN_REPO) and _TRN_REPO not in sys.path:
    sys.path.insert(0, _TRN_REPO)

import concourse.bass as bass  # noqa: E402
import concourse.mybir as mybir  # noqa: E402
import concourse.tile as tile  # noqa: E402
from concourse import bacc  # noqa: E402
from concourse.masks import make_identity  # noqa: E402

F32 = mybir.dt.float32
F32R = mybir.dt.float32r
I8 = mybir.dt.int8

B, T, C, H = 2, 2048, 1024, 16
HD = C // H          # 64
HPC = 4              # heads per core
NCORES = 8
CPC = HPC * HD       # projection channels per core = 256
JSLICE = 512
USE_FP32R = True


def _packing(t):
    nstrip = t // 128
    exts = [128 * (i + 1) for i in range(nstrip)]
    offs = np.cumsum([0] + [128 * e for e in exts]).tolist()
    return nstrip, exts, offs


def build_nc(t=T, use_r=USE_FP32R):
    """Per-core SPMD Bass kernel."""
    nstrip, exts, offs = _packing(t)
    packed = offs[-1]
    nslice = t // JSLICE
    sps = JSLICE // 128          # strips per slice = 4
    nct = C // 128               # 8 input-channel tiles
    nnt = CPC // 128             # 2 projection-output tiles
    nslc_t = t // JSLICE         # 512-wide token slices
    DT = F32R if use_r else F32
    Exp = mybir.ActivationFunctionType.Exp
    add = mybir.AluOpType.add
    mult = mybir.AluOpType.mult

    nc = bacc.Bacc()
    qT = nc.declare_dram_parameter("qT", [C, t], DT, isOutput=False)
    kT = nc.declare_dram_parameter("kT", [C, t], DT, isOutput=False)
    vT = nc.declare_dram_parameter("vT", [C, t], DT, isOutput=False)
    wqT = nc.declare_dram_parameter("wqT", [C, CPC], DT, isOutput=False)
    wkT = nc.declare_dram_parameter("wkT", [C, CPC], DT, isOutput=False)
    wvT = nc.declare_dram_parameter("wvT", [C, CPC], DT, isOutput=False)
    wpT = nc.declare_dram_parameter("wpT", [CPC, C], DT, isOutput=False)
    bqv = nc.declare_dram_parameter("bq", [CPC], F32, isOutput=False)
    bkv = nc.declare_dram_parameter("bk", [CPC], F32, isOutput=False)
    bvv = nc.declare_dram_parameter("bv", [CPC], F32, isOutput=False)
    bias = nc.declare_dram_parameter("bias", [HPC, t, t], F32, isOutput=False)
    sc_out = nc.declare_dram_parameter("sc", [HPC, packed], F32, isOutput=True)
    z_out = nc.declare_dram_parameter("z", [t, C], F32, isOutput=True)

    with tile.TileContext(nc) as tc, ExitStack() as est:
        consts = est.enter_context(tc.tile_pool(name="consts", bufs=1))
        ident = consts.tile([128, 128], F32)
        make_identity(nc, ident[:])
        utri_f = consts.tile([128, 128], F32)
        nc.gpsimd.memset(utri_f[:], 0.0)
        nc.gpsimd.affine_select(
            out=utri_f[:], in_=utri_f[:], compare_op=mybir.AluOpType.is_ge,
            fill=1.0, base=0, pattern=[[-1, 128]], channel_multiplier=1)
        utri = consts.tile([128, 128], I8)
        nc.vector.tensor_copy(utri[:], utri_f[:])
        neginf = consts.tile([128, 128], F32)
        nc.gpsimd.memset(neginf[:], float("-inf"))
        ones4 = consts.tile([128, HPC], F32)
        nc.gpsimd.memset(ones4[:], 1.0)
        bsb = {}
        for nm, src in (("q", bqv), ("k", bkv), ("v", bvv)):
            bsb[nm] = consts.tile([128, nnt], F32, tag=f"b{nm}", name=f"b{nm}")
            for ntl in range(nnt):
                nc.sync.dma_start(
                    bsb[nm][:, ntl:ntl + 1],
                    src[ntl * 128:(ntl + 1) * 128]
                    .rearrange("(p o) -> p o", o=1))

        projp = est.enter_context(tc.tile_pool(name="projT", bufs=2))
        qpT = [projp.tile([128, t], DT, tag="qpT", name="qpT") for _ in range(nnt)]
        kpT = [projp.tile([128, t], DT, tag="kpT", name="kpT") for _ in range(nnt)]
        vaugp = est.enter_context(tc.tile_pool(name="vaug", bufs=nstrip))
        vaug = [vaugp.tile([128, HPC * (HD + 1)], DT, tag="vaug", name="vaug")
                for _ in range(nstrip)]
        wppool = est.enter_context(tc.tile_pool(name="wp", bufs=2))
        wp_sb = [wppool.tile([128, C], DT, tag="wp", name="wp") for _ in range(nnt)]
        for m in range(nnt):
            nc.sync.dma_start(wp_sb[m][:], wpT[m * 128:(m + 1) * 128, :])

        # ---------------- stage A: projections ----------------
        with (
            tc.tile_pool(name="wqkv", bufs=nct) as wpool,
            tc.tile_pool(name="vpT", bufs=2) as vppool,
        ):
            w_sb = {}
            for nm, src in (("q", wqT), ("k", wkT), ("v", wvT)):
                w_sb[nm] = [wpool.tile([128, CPC], DT, tag=f"w{nm}", name=f"w{nm}")
                            for _ in range(nct)]
                for ct in range(nct):
                    nc.sync.dma_start(
                        w_sb[nm][ct][:], src[ct * 128:(ct + 1) * 128, :])
            vpT = [vppool.tile([128, t], F32, tag="vpT", name="vpT") for _ in range(nnt)]

            with (
                tc.tile_pool(name="xT", bufs=3) as xpool,
                tc.tile_pool(name="ppsum", bufs=1, space="PSUM") as ppsum,
            ):
                for nm, xdram, scale in (("q", qT, 0.125), ("k", kT, None),
                                         ("v", vT, None)):
                    psums = [
                        [ppsum.tile([128, JSLICE], F32, tag=f"pp{ntl}_{ts}", name=f"pp{ntl}_{ts}")
                         for ts in range(nslc_t)] for ntl in range(nnt)]
                    for ct in range(nct):
                        xt = xpool.tile([128, t], DT, tag="xT", name="xT")
                        nc.sync.dma_start(
                            xt[:], xdram[ct * 128:(ct + 1) * 128, :])
                        for ntl in range(nnt):
                            lhs = w_sb[nm][ct][:, ntl * 128:(ntl + 1) * 128]
                            for ts in range(nslc_t):
                                nc.tensor.matmul(
                                    psums[ntl][ts][:], lhs,
                                    xt[:, ts * JSLICE:(ts + 1) * JSLICE],
                                    start=(ct == 0), stop=(ct == nct - 1))
                    dst = {"q": qpT, "k": kpT, "v": vpT}[nm]
                    for ntl in range(nnt):
                        b_ap = bsb[nm][:, ntl:ntl + 1]
                        for ts in range(nslc_t):
                            o = dst[ntl][:, ts * JSLICE:(ts + 1) * JSLICE]
                            if scale is not None:
                                nc.vector.tensor_scalar(
                                    o, psums[ntl][ts][:], scalar1=b_ap,
                                    scalar2=scale, op0=add, op1=mult)
                            else:
                                nc.vector.tensor_scalar(
                                    o, psums[ntl][ts][:], scalar1=b_ap,
                                    scalar2=None, op0=add)

            # V_aug: [j, 4*(64+1)] with a ones column per head
            with tc.tile_pool(name="vtp", bufs=2, space="PSUM") as vtp:
                for jt in range(nstrip):
                    va = vaug[jt][:].rearrange("p (h w) -> p h w", w=HD + 1)
                    nc.vector.tensor_copy(va[:, :, HD:HD + 1], ones4[:])
                    for m in range(nnt):
                        ps = vtp.tile([128, 128], F32, tag="vtp", name="vtp")
                        nc.tensor.transpose(
                            ps[:], vpT[m][:, jt * 128:(jt + 1) * 128],
                            ident[:])
                        nc.vector.tensor_copy(
                            va[:, 2 * m:2 * m + 2, 0:HD],
                            ps[:].rearrange("p (h w) -> p h w", w=HD))

        # ---------------- attention ----------------
        spool = est.enter_context(tc.tile_pool(name="scstrip", bufs=2))
        ppool = est.enter_context(tc.tile_pool(name="pstrip", bufs=2))
        slabp = est.enter_context(tc.tile_pool(name="ptslab", bufs=nstrip))
        slab = [slabp.tile([128, JSLICE], DT, tag="slab", name="slab")
                for _ in range(nstrip)]
        ytp = est.enter_context(tc.tile_pool(name="yt", bufs=4))
        r2p = est.enter_context(tc.tile_pool(name="r2", bufs=2))
        rrp = est.enter_context(tc.tile_pool(name="rrow", bufs=2))
        zpool = est.enter_context(tc.tile_pool(name="zsb", bufs=3))
        spsum = est.enter_context(
            tc.tile_pool(name="spsum", bufs=2, space="PSUM"))
        ptps = est.enter_context(
            tc.tile_pool(name="ptpsum", bufs=2, space="PSUM"))
        avz = est.enter_context(tc.tile_pool(name="avz", bufs=2, space="PSUM"))

        for s in range(nslice):
            ypair = [ytp.tile([128, JSLICE], DT, tag="yt", name="yt")
                     for _ in range(HPC // 2)]
            for h in range(HPC):
                pt_idx = h // 2
                prow = slice(64 * (h % 2), 64 * (h % 2) + 64)
                for ii in range(sps):
                    i = s * sps + ii
                    ext = exts[i]
                    tsl = slice(i * 128, (i + 1) * 128)
                    sst = spool.tile([128, t], F32, tag="scstrip", name="scstrip")
                    for js in range(0, ext, JSLICE):
                        w = min(JSLICE, ext - js)
                        sp = spsum.tile([128, JSLICE], F32, tag="spsum", name="spsum")
                        nc.tensor.matmul(
                            sp[:, 0:w], qpT[pt_idx][prow, tsl],
                            kpT[pt_idx][prow, js:js + w],
                            start=True, stop=True)
                        nc.scalar.copy(sst[:, js:js + w], sp[:, 0:w])
                    # bias add during its DMA read, then causal mask
                    nc.gpsimd.dma_start(
                        sst[:, 0:ext], bias[h, tsl, 0:ext], accum_op=add)
                    nc.vector.copy_predicated(
                        sst[:, i * 128:ext], utri[:], neginf[:])
                    nc.sync.dma_start(
                        sc_out[h, offs[i]:offs[i] + 128 * ext]
                        .rearrange("(p w) -> p w", w=ext),
                        sst[:, 0:ext])
                    pst = ppool.tile([128, t], F32, tag="pstrip", name="pstrip")
                    nc.scalar.activation(pst[:, 0:ext], sst[:, 0:ext], Exp)
                    for jt in range(i + 1):
                        pt = ptps.tile([128, 128], F32, tag="ptpsum", name="ptpsum")
                        nc.tensor.transpose(
                            pt[:], pst[:, jt * 128:(jt + 1) * 128], ident[:])
                        nc.vector.tensor_copy(
                            slab[jt][:, ii * 128:(ii + 1) * 128], pt[:])
                # AV (+ row sums via the ones column)
                yp = avz.tile([128, JSLICE], F32, tag="yp", name="yp")
                njt = sps * (s + 1)
                for jt in range(njt):
                    c0 = max(0, (jt - sps * s)) * 128 if jt >= sps * s else 0
                    nc.tensor.matmul(
                        yp[0:HD + 1, c0:JSLICE],
                        vaug[jt][:, (HD + 1) * h:(HD + 1) * (h + 1)],
                        slab[jt][:, c0:JSLICE],
                        start=(jt == 0), stop=(jt == njt - 1))
                rr = rrp.tile([1, JSLICE], F32, tag="rrow", name="rrow")
                nc.vector.reciprocal(rr[0:1, :], yp[HD:HD + 1, :])
                r2 = r2p.tile([64, JSLICE], F32, tag="r2", name="r2")
                nc.gpsimd.partition_broadcast(r2[0:64, :], rr[0:1, :])
                nc.vector.tensor_mul(
                    ypair[pt_idx][prow, :], yp[0:HD, :], r2[0:64, :])
            # output projection for this token slice (partial over heads)
            for tci in range(sps):
                row0 = s * JSLICE + tci * 128
                for oh in range(C // JSLICE):
                    zp_ = avz.tile([128, JSLICE], F32, tag="zp", name="zp")
                    for mt in range(nnt):
                        nc.tensor.matmul(
                            zp_[:], ypair[mt][:, tci * 128:(tci + 1) * 128],
                            wp_sb[mt][:, oh * JSLICE:(oh + 1) * JSLICE],
                            start=(mt == 0), stop=(mt == nnt - 1))
                    zs = zpool.tile([128, JSLICE], F32, tag="zsb", name="zsb")
                    nc.scalar.copy(zs[:], zp_[:])
                    nc.sync.dma_start(
                        z_out[row0:row0 + 128, oh * JSLICE:(oh + 1) * JSLICE],
                        zs[:])

    nc.compile()
    return nc


_NC_CACHE = {}


def _get_nc(t=T, use_r=USE_FP32R):
    key = (t, use_r)
    if key not in _NC_CACHE:
        _NC_CACHE[key] = build_nc(t, use_r)
    return _NC_CACHE[key]


def make_in_maps(q, k, v, attn_bias, Wq, bq, Wk, bk, Wv, bv, Wp, bp):
    """Host-side sharding: per-core input dicts."""
    q, k, v = (np.asarray(a, np.float32) for a in (q, k, v))
    attn_bias = np.asarray(attn_bias, np.float32)
    xT = {b: {} for b in range(B)}
    for b in range(B):
        xT[b]["q"] = np.ascontiguousarray(q[b].T)
        xT[b]["k"] = np.ascontiguousarray(k[b].T)
        xT[b]["v"] = np.ascontiguousarray(v[b].T)
    in_maps = []
    for c in range(NCORES):
        b, g = c // 4, c % 4
        ch = slice(g * CPC, (g + 1) * CPC)
        in_maps.append({
            "qT": xT[b]["q"], "kT": xT[b]["k"], "vT": xT[b]["v"],
            "wqT": np.ascontiguousarray(np.asarray(Wq, np.float32)[ch, :].T),
            "wkT": np.ascontiguousarray(np.asarray(Wk, np.float32)[ch, :].T),
            "wvT": np.ascontiguousarray(np.asarray(Wv, np.float32)[ch, :].T),
            "wpT": np.ascontiguousarray(np.asarray(Wp, np.float32)[:, ch].T),
            "bq": np.ascontiguousarray(np.asarray(bq, np.float32)[ch]),
            "bk": np.ascontiguousarray(np.asarray(bk, np.float32)[ch]),
            "bv": np.ascontiguousarray(np.asarray(bv, np.float32)[ch]),
            "bias": np.ascontiguousarray(
                np.asarray(attn_bias, np.float32)[0, 4 * g:4 * g + 4]),
        })
    return in_maps


def assemble(results, bp):
    """Gather per-core outputs into (y, att_scores)."""
    nstrip, exts, offs = _packing(T)
    att = np.full((B, H, T, T), -np.inf, dtype=np.float32)
    y = np.zeros((B, T, C), dtype=np.float32)
    for c in range(NCORES):
        b, g = c // 4, c % 4
        sc = results[c]["sc"]
        for i in range(nstrip):
            ext = exts[i]
            att[b, 4 * g:4 * g + 4, i * 128:(i + 1) * 128, 0:ext] = (
                sc[:, offs[i]:offs[i] + 128 * ext].reshape(HPC, 128, ext))
        y[b] += results[c]["z"]
    y += np.asarray(bp, np.float32)[None, None, :]
    return y, att


def kernel(q, k, v, attn_bias, Wq, bq, Wk, bk, Wv, bv, Wp, bp):
    from concourse.bass_utils import run_bass_kernel_spmd
    nc = _get_nc()
    in_maps = make_in_maps(q, k, v, attn_bias, Wq, bq, Wk, bk, Wv, bv, Wp, bp)
    res = run_bass_kernel_spmd(nc, in_maps, list(range(NCORES)))
    return assemble(res.results, bp)


# revision 12
# speedup vs baseline: 1.0603x; 1.0603x over previous
"""Causal self-attention Trainium2 kernel (B=2, T=2048, C=1024, H=16).

Returns (y, att_scores) like the reference:
    qh/kh/vh = split_heads(x @ W.T + b)
    att      = (qh @ kh^T) / sqrt(HD)
    scores   = where(tril, att, -inf) + attn_bias          # output 2
    y        = softmax(scores) @ vh -> merge -> @ Wp.T + bp  # output 1

Sharding (8 cores): core c owns batch b = c//4 and heads 4*(c%4)..+4
(data parallel on B x tensor parallel on H).  Each core runs its 4 heads'
projections (host-sliced weight rows), full causal attention, and a
partial output projection; the host sums the four per-batch partials
(the output projection contracts over head channels) and adds bp.

Device-side notes:
  - q/k/v ship pre-transposed [C, T] so the C contraction lies on SBUF
    partitions; weights ship pre-sliced/pre-transposed.
  - scores are written packed (lower-triangle 128-row strips only); the
    constant -inf upper triangle is filled host-side.  attn_bias is added
    during its own DMA (SWDGE accum_op=add), costing no engine time.
  - P = exp(scores) rides exp(-inf)=0 for the causal mask; P tiles are
    transposed 128x128 on the PE to feed P^T into the AV matmul.  V is
    augmented with a ones column so the AV matmul also emits the softmax
    row sums (unnormalized-softmax trick); 1/l is broadcast and applied
    when copying y^T out of PSUM, right before the output projection.
  - matmuls run in float32r (PE truncates fp32 operands to ~FP22; same
    read bytes, 4x the throughput of true fp32).  Set USE_FP32R=False to
    fall back to exact fp32 matmuls.
"""

import os
import sys
from contextlib import ExitStack

import numpy as np

_TRN_REPO = "/opt/trn_rl_repo"
if os.path.isdir(_TRN_REPO) and _TRN_REPO not in sys.path:
    sys.path.insert(0, _TRN_REPO)

import concourse.bass as bass  # noqa: E402
import concourse.mybir as mybir  # noqa: E402
import concourse.tile as tile  # noqa: E402
from concourse import bacc  # noqa: E402
from concourse.masks import make_identity  # noqa: E402

F32 = mybir.dt.float32
F32R = mybir.dt.float32r
I8 = mybir.dt.int8

B, T, C, H = 2, 2048, 1024, 16
HD = C // H          # 64
HPC = 4              # heads per core
NCORES = 8
CPC = HPC * HD       # projection channels per core = 256
JSLICE = 512
USE_FP32R = True


def _packing(t):
    nstrip = t // 128
    exts = [128 * (i + 1) for i in range(nstrip)]
    offs = np.cumsum([0] + [128 * e for e in exts]).tolist()
    return nstrip, exts, offs


def build_nc(t=T, use_r=USE_FP32R, ablate=frozenset()):
    """Per-core SPMD Bass kernel."""
    nstrip, exts, offs = _packing(t)
    packed = offs[-1]
    nslice = t // JSLICE
    sps = JSLICE // 128          # strips per slice = 4
    nct = C // 128               # 8 input-channel tiles
    nnt = CPC // 128             # 2 projection-output tiles
    nslc_t = t // JSLICE         # 512-wide token slices
    DT = F32R if use_r else F32
    Exp = mybir.ActivationFunctionType.Exp
    add = mybir.AluOpType.add
    mult = mybir.AluOpType.mult

    nc = bacc.Bacc()
    qT = nc.declare_dram_parameter("qT", [C, t], DT, isOutput=False)
    kT = nc.declare_dram_parameter("kT", [C, t], DT, isOutput=False)
    vT = nc.declare_dram_parameter("vT", [C, t], DT, isOutput=False)
    wqT = nc.declare_dram_parameter("wqT", [C, CPC], DT, isOutput=False)
    wkT = nc.declare_dram_parameter("wkT", [C, CPC], DT, isOutput=False)
    wvT = nc.declare_dram_parameter("wvT", [C, CPC], DT, isOutput=False)
    wpT = nc.declare_dram_parameter("wpT", [CPC, C], DT, isOutput=False)
    bqv = nc.declare_dram_parameter("bq", [CPC], F32, isOutput=False)
    bkv = nc.declare_dram_parameter("bk", [CPC], F32, isOutput=False)
    bvv = nc.declare_dram_parameter("bv", [CPC], F32, isOutput=False)
    bias = nc.declare_dram_parameter("bias", [HPC, t, t], F32, isOutput=False)
    sc_out = nc.declare_dram_parameter("sc", [HPC, packed], F32, isOutput=True)
    z_out = nc.declare_dram_parameter("z", [t, C], F32, isOutput=True)

    with tile.TileContext(nc) as tc, ExitStack() as est:
        consts = est.enter_context(tc.tile_pool(name="consts", bufs=1))
        ident = consts.tile([128, 128], F32)
        make_identity(nc, ident[:])
        utri_f = consts.tile([128, 128], F32)
        nc.gpsimd.memset(utri_f[:], 0.0)
        nc.gpsimd.affine_select(
            out=utri_f[:], in_=utri_f[:], compare_op=mybir.AluOpType.is_ge,
            fill=1.0, base=0, pattern=[[-1, 128]], channel_multiplier=1)
        utri = consts.tile([128, 128], I8)
        nc.vector.tensor_copy(utri[:], utri_f[:])
        neginf = consts.tile([128, 128], F32)
        nc.gpsimd.memset(neginf[:], float("-inf"))
        ones4 = consts.tile([128, HPC], F32)
        nc.gpsimd.memset(ones4[:], 1.0)
        bsb = {}
        for nm, src in (("q", bqv), ("k", bkv), ("v", bvv)):
            bsb[nm] = consts.tile([128, nnt], F32, tag=f"b{nm}", name=f"b{nm}")
            for ntl in range(nnt):
                nc.sync.dma_start(
                    bsb[nm][:, ntl:ntl + 1],
                    src[ntl * 128:(ntl + 1) * 128]
                    .rearrange("(p o) -> p o", o=1))

        projp = est.enter_context(tc.tile_pool(name="projT", bufs=2))
        qpT = [projp.tile([128, t], DT, tag="qpT", name="qpT") for _ in range(nnt)]
        kpT = [projp.tile([128, t], DT, tag="kpT", name="kpT") for _ in range(nnt)]
        vaugp = est.enter_context(tc.tile_pool(name="vaug", bufs=nstrip))
        vaug = [vaugp.tile([128, HPC * (HD + 1)], DT, tag="vaug", name="vaug")
                for _ in range(nstrip)]
        wppool = est.enter_context(tc.tile_pool(name="wp", bufs=2))
        wp_sb = [wppool.tile([128, C], DT, tag="wp", name="wp") for _ in range(nnt)]
        for m in range(nnt):
            nc.sync.dma_start(wp_sb[m][:], wpT[m * 128:(m + 1) * 128, :])

        # ---------------- stage A: projections ----------------
        with (
            tc.tile_pool(name="wqkv", bufs=nct) as wpool,
            tc.tile_pool(name="vpT", bufs=2) as vppool,
        ):
            w_sb = {}
            for nm, src in (("q", wqT), ("k", wkT), ("v", wvT)):
                w_sb[nm] = [wpool.tile([128, CPC], DT, tag=f"w{nm}", name=f"w{nm}")
                            for _ in range(nct)]
                for ct in range(nct):
                    nc.sync.dma_start(
                        w_sb[nm][ct][:], src[ct * 128:(ct + 1) * 128, :])
            vpT = [vppool.tile([128, t], F32, tag="vpT", name="vpT") for _ in range(nnt)]

            with (
                tc.tile_pool(name="xT", bufs=3) as xpool,
                tc.tile_pool(name="ppsum", bufs=1, space="PSUM") as ppsum,
            ):
                for nm, xdram, scale in (("q", qT, 0.125), ("k", kT, None),
                                         ("v", vT, None)):
                    psums = [
                        [ppsum.tile([128, JSLICE], F32, tag=f"pp{ntl}_{ts}", name=f"pp{ntl}_{ts}")
                         for ts in range(nslc_t)] for ntl in range(nnt)]
                    for ct in range(nct):
                        xt = xpool.tile([128, t], DT, tag="xT", name="xT")
                        nc.sync.dma_start(
                            xt[:], xdram[ct * 128:(ct + 1) * 128, :])
                        for ntl in range(nnt):
                            lhs = w_sb[nm][ct][:, ntl * 128:(ntl + 1) * 128]
                            for ts in range(nslc_t):
                                nc.tensor.matmul(
                                    psums[ntl][ts][:], lhs,
                                    xt[:, ts * JSLICE:(ts + 1) * JSLICE],
                                    start=(ct == 0), stop=(ct == nct - 1))
                    dst = {"q": qpT, "k": kpT, "v": vpT}[nm]
                    for ntl in range(nnt):
                        b_ap = bsb[nm][:, ntl:ntl + 1]
                        for ts in range(nslc_t):
                            o = dst[ntl][:, ts * JSLICE:(ts + 1) * JSLICE]
                            if scale is not None:
                                nc.vector.tensor_scalar(
                                    o, psums[ntl][ts][:], scalar1=b_ap,
                                    scalar2=scale, op0=add, op1=mult)
                            else:
                                nc.vector.tensor_scalar(
                                    o, psums[ntl][ts][:], scalar1=b_ap,
                                    scalar2=None, op0=add)

            # V_aug: [j, 4*(64+1)] with a ones column per head
            with tc.tile_pool(name="vtp", bufs=2, space="PSUM") as vtp:
                for jt in range(nstrip):
                    va = vaug[jt][:].rearrange("p (h w) -> p h w", w=HD + 1)
                    nc.vector.tensor_copy(va[:, :, HD:HD + 1], ones4[:])
                    for m in range(nnt):
                        ps = vtp.tile([128, 128], F32, tag="vtp", name="vtp")
                        nc.tensor.transpose(
                            ps[:], vpT[m][:, jt * 128:(jt + 1) * 128],
                            ident[:])
                        nc.vector.tensor_copy(
                            va[:, 2 * m:2 * m + 2, 0:HD],
                            ps[:].rearrange("p (h w) -> p h w", w=HD))

        # ---------------- attention ----------------
        spool = est.enter_context(tc.tile_pool(name="scstrip", bufs=2))
        ppool = est.enter_context(tc.tile_pool(name="pstrip", bufs=2))
        slabp = est.enter_context(tc.tile_pool(name="ptslab", bufs=nstrip))
        slab = [slabp.tile([128, JSLICE], DT, tag="slab", name="slab")
                for _ in range(nstrip)]
        ytp = est.enter_context(tc.tile_pool(name="yt", bufs=4))
        r2p = est.enter_context(tc.tile_pool(name="r2", bufs=2))
        rrp = est.enter_context(tc.tile_pool(name="rrow", bufs=2))
        zpool = est.enter_context(tc.tile_pool(name="zsb", bufs=3))
        spsum = est.enter_context(
            tc.tile_pool(name="spsum", bufs=2, space="PSUM"))
        ptps = est.enter_context(
            tc.tile_pool(name="ptpsum", bufs=2, space="PSUM"))
        avz = est.enter_context(tc.tile_pool(name="avz", bufs=2, space="PSUM"))

        for s in range(nslice):
            ypair = [ytp.tile([128, JSLICE], DT, tag="yt", name="yt")
                     for _ in range(HPC // 2)]
            for h in range(HPC):
                pt_idx = h // 2
                prow = slice(64 * (h % 2), 64 * (h % 2) + 64)
                for ii in range(sps):
                    i = s * sps + ii
                    ext = exts[i]
                    tsl = slice(i * 128, (i + 1) * 128)
                    sst = spool.tile([128, t], F32, tag="scstrip", name="scstrip")
                    for js in range(0, ext, JSLICE):
                        w = min(JSLICE, ext - js)
                        sp = spsum.tile([128, JSLICE], F32, tag="spsum", name="spsum")
                        nc.tensor.matmul(
                            sp[:, 0:w], qpT[pt_idx][prow, tsl],
                            kpT[pt_idx][prow, js:js + w],
                            start=True, stop=True)
                        nc.scalar.copy(sst[:, js:js + w], sp[:, 0:w])
                    # bias add during its DMA read, then causal mask
                    if "bias" not in ablate:
                        nc.gpsimd.dma_start(
                            sst[:, 0:ext], bias[h, tsl, 0:ext], accum_op=add)
                    nc.vector.copy_predicated(
                        sst[:, i * 128:ext], utri[:], neginf[:])
                    if "scout" not in ablate:
                        nc.sync.dma_start(
                            sc_out[h, offs[i]:offs[i] + 128 * ext]
                            .rearrange("(p w) -> p w", w=ext),
                            sst[:, 0:ext])
                    if "tail" in ablate:
                        continue
                    pst = ppool.tile([128, t], F32, tag="pstrip", name="pstrip")
                    nc.scalar.activation(pst[:, 0:ext], sst[:, 0:ext], Exp)
                    for jt in range(i + 1):
                        pt = ptps.tile([128, 128], F32, tag="ptpsum", name="ptpsum")
                        nc.tensor.transpose(
                            pt[:], pst[:, jt * 128:(jt + 1) * 128], ident[:])
                        nc.vector.tensor_copy(
                            slab[jt][:, ii * 128:(ii + 1) * 128], pt[:])
                if "tail" in ablate:
                    continue
                # AV (+ row sums via the ones column)
                yp = avz.tile([128, JSLICE], F32, tag="yp", name="yp")
                njt = sps * (s + 1)
                for jt in range(njt):
                    c0 = max(0, (jt - sps * s)) * 128 if jt >= sps * s else 0
                    nc.tensor.matmul(
                        yp[0:HD + 1, c0:JSLICE],
                        vaug[jt][:, (HD + 1) * h:(HD + 1) * (h + 1)],
                        slab[jt][:, c0:JSLICE],
                        start=(jt == 0), stop=(jt == njt - 1))
                rr = rrp.tile([1, JSLICE], F32, tag="rrow", name="rrow")
                nc.vector.reciprocal(rr[0:1, :], yp[HD:HD + 1, :])
                r2 = r2p.tile([64, JSLICE], F32, tag="r2", name="r2")
                nc.gpsimd.partition_broadcast(r2[0:64, :], rr[0:1, :])
                nc.vector.tensor_mul(
                    ypair[pt_idx][prow, :], yp[0:HD, :], r2[0:64, :])
            # output projection for this token slice (partial over heads)
            for tci in range(sps if "tail" not in ablate else 0):
                row0 = s * JSLICE + tci * 128
                for oh in range(C // JSLICE):
                    zp_ = avz.tile([128, JSLICE], F32, tag="zp", name="zp")
                    for mt in range(nnt):
                        nc.tensor.matmul(
                            zp_[:], ypair[mt][:, tci * 128:(tci + 1) * 128],
                            wp_sb[mt][:, oh * JSLICE:(oh + 1) * JSLICE],
                            start=(mt == 0), stop=(mt == nnt - 1))
                    zs = zpool.tile([128, JSLICE], F32, tag="zsb", name="zsb")
                    nc.scalar.copy(zs[:], zp_[:])
                    nc.sync.dma_start(
                        z_out[row0:row0 + 128, oh * JSLICE:(oh + 1) * JSLICE],
                        zs[:])

    nc.compile()
    return nc


_NC_CACHE = {}


def _get_nc(t=T, use_r=USE_FP32R):
    key = (t, use_r)
    if key not in _NC_CACHE:
        _NC_CACHE[key] = build_nc(t, use_r)
    return _NC_CACHE[key]


def make_in_maps(q, k, v, attn_bias, Wq, bq, Wk, bk, Wv, bv, Wp, bp):
    """Host-side sharding: per-core input dicts."""
    q, k, v = (np.asarray(a, np.float32) for a in (q, k, v))
    attn_bias = np.asarray(attn_bias, np.float32)
    xT = {b: {} for b in range(B)}
    for b in range(B):
        xT[b]["q"] = np.ascontiguousarray(q[b].T)
        xT[b]["k"] = np.ascontiguousarray(k[b].T)
        xT[b]["v"] = np.ascontiguousarray(v[b].T)
    in_maps = []
    for c in range(NCORES):
        b, g = c // 4, c % 4
        ch = slice(g * CPC, (g + 1) * CPC)
        in_maps.append({
            "qT": xT[b]["q"], "kT": xT[b]["k"], "vT": xT[b]["v"],
            "wqT": np.ascontiguousarray(np.asarray(Wq, np.float32)[ch, :].T),
            "wkT": np.ascontiguousarray(np.asarray(Wk, np.float32)[ch, :].T),
            "wvT": np.ascontiguousarray(np.asarray(Wv, np.float32)[ch, :].T),
            "wpT": np.ascontiguousarray(np.asarray(Wp, np.float32)[:, ch].T),
            "bq": np.ascontiguousarray(np.asarray(bq, np.float32)[ch]),
            "bk": np.ascontiguousarray(np.asarray(bk, np.float32)[ch]),
            "bv": np.ascontiguousarray(np.asarray(bv, np.float32)[ch]),
            "bias": np.ascontiguousarray(
                np.asarray(attn_bias, np.float32)[0, 4 * g:4 * g + 4]),
        })
    return in_maps


def assemble(results, bp):
    """Gather per-core outputs into (y, att_scores)."""
    nstrip, exts, offs = _packing(T)
    att = np.full((B, H, T, T), -np.inf, dtype=np.float32)
    y = np.zeros((B, T, C), dtype=np.float32)
    for c in range(NCORES):
        b, g = c // 4, c % 4
        sc = results[c]["sc"]
        for i in range(nstrip):
            ext = exts[i]
            att[b, 4 * g:4 * g + 4, i * 128:(i + 1) * 128, 0:ext] = (
                sc[:, offs[i]:offs[i] + 128 * ext].reshape(HPC, 128, ext))
        y[b] += results[c]["z"]
    y += np.asarray(bp, np.float32)[None, None, :]
    return y, att


def kernel(q, k, v, attn_bias, Wq, bq, Wk, bk, Wv, bv, Wp, bp):
    from concourse.bass_utils import run_bass_kernel_spmd
    nc = _get_nc()
    in_maps = make_in_maps(q, k, v, attn_bias, Wq, bq, Wk, bk, Wv, bv, Wp, bp)
    res = run_bass_kernel_spmd(nc, in_maps, list(range(NCORES)))
    return assemble(res.results, bp)
